# revision 24
# baseline (speedup 1.0000x reference)
"""Multi-head causal attention (B=2, S=2048, D=1024, H=16) on 8 TRN2 NeuronCores.

Sharding: core c -> batch c//4, head-quarter c%4 (4 heads = 256 head dims).
Each core runs the full pipeline for its (batch, 4 heads):
  QKV projections -> causal softmax(QK^T/8) -> PV -> partial out-projection.
Host pre-transposes x / weight shards (so every DMA is contiguous) and
sums the 4 row-sharded out-projection partials per batch + bias.

Engine split: PE matmuls (projections fp32r, scores fp32r pair-tiled on
rows 0-63/64-127, PV bf16), ACT exclusively exp (one activation per
(i, head-pair) over a 2-bank PSUM score tile), DVE all PSUM->SBUF copies +
normalize, GpSimd(Pool) causal-mask multiplies. Softmax skips
max-subtraction (scores bounded ~ +-4); row sums come free as a 65th ones
row in V. Diagonal blocks trim matmul N (scores >=256 for fp32r rate,
PV exact since bf16 runs full rate at any N).
"""

import sys

import numpy as np

if "/opt/trn_rl_repo" not in sys.path:
    sys.path.insert(0, "/opt/trn_rl_repo")

import concourse.bass as bass
import concourse.mybir as mybir
import concourse.tile as tile
from concourse import library_config
from concourse.bass import ts
from concourse.bass_utils import run_bass_kernel_spmd

P = 128          # partitions
S = 2048         # sequence length
DD = 1024        # model dim
DC = DD // P     # d-model chunks (8)
E = 256          # head dims per core (4 heads x 64)
H4 = 4           # heads per core
HD = 64
NQ = 4           # q chunks of 512
QC = 512
KT = S // P      # k tiles (16)
FD = 512         # matmul free dim

F32 = mybir.dt.float32
FR = mybir.dt.float32r
BF = mybir.dt.bfloat16
EXP = mybir.ActivationFunctionType.Exp
MUL = mybir.AluOpType.mult


def _emit(tc, nc, xT_d, wq_d, wk_d, wv_d, wo_d, msk_d, out_d):
    with (
        tc.tile_pool(name="const", bufs=1) as const,
        tc.tile_pool(name="attn", bufs=4) as attn_pool,
        tc.tile_pool(name="small", bufs=2) as small,
        tc.tile_pool(name="ostage", bufs=2) as ostage,
        tc.tile_pool(name="pmm", bufs=2, space="PSUM") as pmm,
        tc.tile_pool(name="pacc", bufs=2, space="PSUM") as pacc,
        tc.tile_pool(name="psc", bufs=2, space="PSUM") as psc,
    ):
        xT = const.tile([P, DC, S], BF)
        wq = const.tile([P, DC, E], BF)
        wk = const.tile([P, DC, E], BF)
        wv = const.tile([P, DC, E], BF)
        wo = const.tile([P, 2, DD], FR)
        msk = const.tile([P, 2 * P], BF)  # [zeros(128) | upper-tri(128)]
        qT = const.tile([P, 2, S], BF)
        kT = const.tile([P, 2, S], BF)
        vS = const.tile([P, KT, H4, HD + 1], BF)
        cT = const.tile([P, 2, S], FR)

        # per-chunk loads interleaved so the first projection matmuls can
        # start as soon as (wq_c, x_c) pairs land
        nc.sync.dma_start(msk[:], msk_d[:])
        for c in range(DC):
            nc.sync.dma_start(wq[:, c], wq_d[:, c])
            nc.sync.dma_start(xT[:, c, ts(0, QC)], xT_d[:, c, ts(0, QC)])
        nc.sync.dma_start(wk[:], wk_d[:])
        nc.sync.dma_start(wv[:], wv_d[:])
        # remaining x^T in (j, c) blocks
        for j in range(1, NQ):
            for c in range(DC):
                nc.sync.dma_start(xT[:, c, ts(j, QC)], xT_d[:, c, ts(j, QC)])
        nc.sync.dma_start(wo[:], wo_d[:])

        tri = msk[:, P:]        # [128, 128] upper-tri ones (q >= k)
        zt = msk[:, :]          # [128, 256] zeros | tri

        # ones column of V_ext (row sums of exp-scores come out of PV)
        nc.gpsimd.memset(vS[:, :, :, HD], 1.0)
        # f32r ones row for the rowsum-broadcast matmul (Memset can't encode
        # f32r; a copy can round to it)
        onesf = const.tile([1, HD], F32)
        nc.vector.memset(onesf[:], 1.0)
        ones64 = const.tile([1, HD], FR)
        nc.vector.tensor_copy(ones64[:], onesf[:])

        def emit_proj(j):
            # QKV projections for q-chunk j; et=0 (head-pair 0) first for both
            # Q and K so attention hp=0 can begin before et=1 lands
            for et in range(2):
                for w_s, dst in ((wq, qT), (wk, kT)):
                    ps = pmm.tile([P, FD], F32, tag="mm", name="ps_proj")
                    for c in range(DC):
                        nc.tensor.matmul(
                            ps[:],
                            lhsT=w_s[:, c, ts(et, P)],
                            rhs=xT[:, c, ts(j, QC)],
                            start=(c == 0),
                            stop=(c == DC - 1),
                        )
                    nc.vector.tensor_copy(dst[:, et, ts(j, QC)], ps[:])
            for nt in range(4 * j, 4 * j + 4):
                psv = pmm.tile([P, FD], F32, tag="mm", name="ps_v")
                for c in range(DC):
                    nc.tensor.matmul(
                        psv[:, :E],
                        lhsT=xT[:, c, ts(nt, P)],
                        rhs=wv[:, c, :],
                        start=(c == 0),
                        stop=(c == DC - 1),
                    )
                nc.vector.tensor_copy(
                    vS[:, nt, :, 0:HD],
                    psv[:, :E].rearrange("p (h d) -> p h d", h=H4),
                )

        emit_proj(0)
        for j in range(NQ):
            # ---- attention for q-chunk j, heads processed in pairs ----
            nk = 4 * (j + 1)
            for hp in range(2):
                h0, h1 = 2 * hp, 2 * hp + 1
                pv0 = pacc.tile([HD + 1, QC], F32, tag="pv", name="pv0")
                pv1 = pacc.tile([HD + 1, QC], F32, tag="pv", name="pv1")
                for i in range(nk):
                    coff = P * (i - 4 * j)  # <0 for full (non-diagonal) tiles
                    cs = max(coff, 0)  # bf16 matmuls: full rate at any N
                    sc = psc.tile([P, 2, QC], F32, tag="sc", name="sc")
                    qs = slice(QC * j + cs, QC * (j + 1))
                    nc.tensor.matmul(
                        sc[:, 0, cs:QC],
                        lhsT=kT[0:HD, hp, ts(i, P)],
                        rhs=qT[0:HD, hp, qs],
                        start=True,
                        stop=True,
                    )
                    nc.tensor.matmul(
                        sc[:, 1, cs:QC],
                        lhsT=kT[HD:P, hp, ts(i, P)],
                        rhs=qT[HD:P, hp, qs],
                        start=True,
                        stop=True,
                    )
                    at = attn_pool.tile([P, 2, QC], BF, tag="at", name="at")
                    nc.scalar.activation(at[:, :, cs:QC], sc[:, :, cs:QC], EXP)
                    if coff >= 0:
                        # causal mask on the diagonal 128-block
                        for hh in range(2):
                            nc.gpsimd.tensor_tensor(
                                at[:, hh, coff:coff + P],
                                at[:, hh, coff:coff + P],
                                tri[:],
                                MUL,
                            )
                    pn = cs  # PV trim
                    for hh, pv in ((0, pv0), (1, pv1)):
                        nc.tensor.matmul(
                            pv[:, pn:QC],
                            lhsT=vS[:, i, 2 * hp + hh, :],
                            rhs=at[:, hh, pn:QC],
                            start=(i == 0),
                            stop=(i == nk - 1),
                            skip_group_check=True,
                        )
                # one 2-bank PSUM tile for both heads' rowsum broadcasts so
                # the other score slot stays free for the next i-loop
                bcp = psc.tile([P, 2, QC], F32, tag="sc", name="bcp")
                for hh, pv in ((0, pv0), (1, pv1)):
                    rec_fr = small.tile([1, QC], FR, tag="recf", name="rec_fr")
                    # f32r is full-width fp32; only the PE treats it specially
                    with nc.allow_low_precision(reason="f32r == fp32 width"):
                        nc.vector.reciprocal(rec_fr[:], pv[HD:HD + 1, :])
                    # broadcast 1/rowsum across 64 partitions via a rank-1
                    # ones-matmul (PE)
                    nc.tensor.matmul(
                        bcp[0:HD, hh, :], lhsT=ones64[:], rhs=rec_fr[:],
                        start=True, stop=True,
                    )
                for hh, pv in ((0, pv0), (1, pv1)):
                    po = HD * hh
                    bc = small.tile([HD, QC], F32, tag="bc", name="bc")
                    nc.vector.tensor_copy(bc[:], bcp[0:HD, hh, :])
                    nc.vector.tensor_tensor(
                        cT[po:po + HD, hp, ts(j, QC)], pv[0:HD, :], bc[:], MUL
                    )

            # ---- next chunk's projections first: their PSUM slots queue
            # ahead of out-proj's, so the scheduler can slide them into the
            # ACT-bound attention gaps of this chunk ----
            if j + 1 < NQ:
                emit_proj(j + 1)

            # ---- out-projection for the finished n-chunk ----
            for nt in range(4 * j, 4 * j + 4):
                for fc in range(2):
                    po = pmm.tile([P, FD], F32, tag="mm", name="ps_out")
                    for c in range(2):
                        nc.tensor.matmul(
                            po[:],
                            lhsT=cT[:, c, ts(nt, P)],
                            rhs=wo[:, c, ts(fc, FD)],
                            start=(c == 0),
                            stop=(c == 1),
                        )
                    ob = ostage.tile([P, FD], F32, tag="ob", name="ob")
                    nc.vector.tensor_copy(ob[:], po[:])
                    nc.sync.dma_start(out_d[ts(nt, P), ts(fc, FD)], ob[:])


def _split_multi_waits(nc):
    """The TRN2 instruction encoding carries ONE sync-wait slot; this walrus
    build rejects instructions with more. Hoist extra waits onto standalone
    EventSemaphore instructions immediately before (same engine queue, same
    semantics)."""
    n = 0
    for f in nc.m.functions:
        for b in f.blocks:
            out = []
            for i in list(b.instructions):
                si = i.sync_info
                if si is not None and len(si.on_wait) > 1:
                    waits = list(si.on_wait)
                    for w in waits[:-1]:
                        n += 1
                        out.append(
                            mybir.InstEventSemaphore(
                                name=f"I-wsplit{n}",
                                engine=i.engine,
                                ins=[],
                                outs=[],
                                sync_info=mybir.SyncInfo(on_wait=[w], on_update=[]),
                            )
                        )
                    i.sync_info = mybir.SyncInfo(
                        on_wait=[waits[-1]], on_update=list(si.on_update)
                    )
                out.append(i)
            b.instructions = out


def build_nc(split_waits=True):
    nc = bass.Bass("TRN2", target_bir_lowering=False, debug=False)
    xT_d = nc.dram_tensor("xT", [P, DC, S], BF, kind="ExternalInput").ap()
    wq_d = nc.dram_tensor("wqT", [P, DC, E], BF, kind="ExternalInput").ap()
    wk_d = nc.dram_tensor("wkT", [P, DC, E], BF, kind="ExternalInput").ap()
    wv_d = nc.dram_tensor("wvT", [P, DC, E], BF, kind="ExternalInput").ap()
    wo_d = nc.dram_tensor("woT", [P, 2, DD], FR, kind="ExternalInput").ap()
    msk_d = nc.dram_tensor("msk", [P, 2 * P], BF, kind="ExternalInput").ap()
    out_d = nc.dram_tensor("out", [S, DD], F32, kind="ExternalOutput").ap()
    with tile.TileContext(nc) as tc:
        _emit(tc, nc, xT_d, wq_d, wk_d, wv_d, wo_d, msk_d, out_d)
    if split_waits:
        _split_multi_waits(nc)
    return nc


def _strip(a, chunks):
    """[D, N] -> [128, D//128, N] with partition-major layout, contiguous."""
    d, n = a.shape
    return np.ascontiguousarray(
        a.reshape(chunks, P, n).transpose(1, 0, 2), dtype=np.float32
    )


import ml_dtypes  # noqa: E402


def make_in_maps(x, Wq, Wk, Wv, Wo):

    tri = np.triu(np.ones((P, P), np.float32))
    msk = np.concatenate([np.zeros((P, P), np.float32), tri], axis=1).astype(
        ml_dtypes.bfloat16
    )
    in_maps = []
    for c in range(8):
        b, g = c // 4, c % 4
        sl = slice(E * g, E * (g + 1))
        in_maps.append(
            {
                "xT": _strip(x[b].T.astype(np.float32), DC).astype(
                    ml_dtypes.bfloat16
                ),
                "wqT": _strip((Wq[sl, :] * 0.125).T.astype(np.float32), DC)
                .astype(ml_dtypes.bfloat16),
                "wkT": _strip(Wk[sl, :].T.astype(np.float32), DC).astype(
                    ml_dtypes.bfloat16
                ),
                "wvT": _strip(Wv[sl, :].T.astype(np.float32), DC).astype(
                    ml_dtypes.bfloat16
                ),
                "woT": _strip(Wo[:, sl].T.astype(np.float32), 2),
                "msk": msk,
            }
        )
    return in_maps


def kernel(x, Wq, Wk, Wv, Wo, bo, _run_kwargs=None):
    x, Wq, Wk, Wv, Wo, bo = (
        np.asarray(a, dtype=np.float32) for a in (x, Wq, Wk, Wv, Wo, bo)
    )
    nc = build_nc()
    in_maps = make_in_maps(x, Wq, Wk, Wv, Wo)
    res = run_bass_kernel_spmd(
        nc, in_maps, core_ids=list(range(8)), **(_run_kwargs or {})
    )
    out = np.zeros((2, S, DD), dtype=np.float32)
    for c in range(8):
        out[c // 4] += res.results[c]["out"]
    out += bo[None, None, :]
    if _run_kwargs:
        kernel.last_results = res
    return out


# revision 40
# speedup vs baseline: 1.0933x; 1.0933x over previous
"""Multi-head causal attention (B=2, S=2048, D=1024, H=16) on 8 TRN2 NeuronCores.

Sharding: core c -> batch c//4, head-quarter c%4 (4 heads = 256 head dims).
Each core runs the full pipeline for its (batch, 4 heads):
  QKV projections -> causal softmax(QK^T/8) -> PV -> partial out-projection.
Host pre-transposes x / weight shards (so every DMA is contiguous) and
sums the 4 row-sharded out-projection partials per batch + bias.

Engine split: PE matmuls (QKV projections / scores / PV in bf16 at full
PE rate, scores pair-tiled on rows 0-63/64-127 so each head pair runs
concurrently, out-projection fp32r), ACT exclusively exp (one activation
per (i, head-pair) spanning a 2-bank PSUM score tile), DVE all
PSUM->SBUF copies + softmax normalize, GpSimd(Pool) causal-mask
multiplies. Softmax skips max-subtraction (scores bounded ~ +-4); row
sums come free as a 65th ones-row in V. Diagonal blocks trim matmul N
exactly (bf16 streams full rate at any N). Emission order is pipelined:
chunk j+1's projections are emitted before chunk j's out-projection so
their PSUM slots queue first and the scheduler slides them into the
ACT-bound attention gaps; DMAs are batched (~0.7us SP issue cost each).
"""

import sys

import numpy as np

if "/opt/trn_rl_repo" not in sys.path:
    sys.path.insert(0, "/opt/trn_rl_repo")

import concourse.bass as bass
import concourse.mybir as mybir
import concourse.tile as tile
from concourse import library_config
from concourse.bass import ts
from concourse.bass_utils import run_bass_kernel_spmd

P = 128          # partitions
S = 2048         # sequence length
DD = 1024        # model dim
DC = DD // P     # d-model chunks (8)
E = 256          # head dims per core (4 heads x 64)
H4 = 4           # heads per core
HD = 64
NQ = 4           # q chunks of 512
QC = 512
KT = S // P      # k tiles (16)
FD = 512         # matmul free dim

F32 = mybir.dt.float32
FR = mybir.dt.float32r
BF = mybir.dt.bfloat16
EXP = mybir.ActivationFunctionType.Exp
MUL = mybir.AluOpType.mult


def _emit(tc, nc, xT_d, wq_d, wk_d, wv_d, wo_d, msk_d, out_d):
    with (
        tc.tile_pool(name="const", bufs=1) as const,
        tc.tile_pool(name="attn", bufs=6) as attn_pool,
        tc.tile_pool(name="small", bufs=2) as small,
        tc.tile_pool(name="ostage", bufs=2) as ostage,
        tc.tile_pool(name="pmm", bufs=2, space="PSUM") as pmm,
        tc.tile_pool(name="pacc", bufs=2, space="PSUM") as pacc,
        tc.tile_pool(name="psc", bufs=2, space="PSUM") as psc,
    ):
        xT = const.tile([P, DC, S], BF)
        wq = const.tile([P, DC, E], BF)
        wk = const.tile([P, DC, E], BF)
        wv = const.tile([P, DC, E], BF)
        wo = const.tile([P, 2, DD], FR)
        msk = const.tile([P, 2 * P], BF)  # [zeros(128) | upper-tri(128)]
        qT = const.tile([P, 2, S], BF)
        kT = const.tile([P, 2, S], BF)
        vS = const.tile([P, KT, H4, HD + 1], BF)
        cT = const.tile([P, 2, S], FR)

        # DMA issue on SP costs ~0.7us per dma_start, so batch: halves of
        # (wq, xT j=0) first so the first projection matmuls start early
        nc.sync.dma_start(wq[:, 0:4], wq_d[:, 0:4])
        nc.sync.dma_start(xT[:, 0:4, ts(0, QC)], xT_d[:, 0:4, ts(0, QC)])
        nc.sync.dma_start(wq[:, 4:8], wq_d[:, 4:8])
        nc.sync.dma_start(xT[:, 4:8, ts(0, QC)], xT_d[:, 4:8, ts(0, QC)])
        nc.sync.dma_start(wk[:], wk_d[:])
        nc.sync.dma_start(msk[:], msk_d[:])
        nc.sync.dma_start(wv[:], wv_d[:])
        # remaining x^T per q-chunk
        for j in range(1, NQ):
            nc.sync.dma_start(xT[:, :, ts(j, QC)], xT_d[:, :, ts(j, QC)])
        nc.sync.dma_start(wo[:], wo_d[:])

        tri = msk[:, P:]        # [128, 128] upper-tri ones (q >= k)
        zt = msk[:, :]          # [128, 256] zeros | tri

        # ones column of V_ext (row sums of exp-scores come out of PV)
        nc.gpsimd.memset(vS[:, :, :, HD], 1.0)
        # f32r ones row for the rowsum-broadcast matmul (Memset can't encode
        # f32r; a copy can round to it)
        onesf = const.tile([1, HD], F32)
        nc.vector.memset(onesf[:], 1.0)
        ones64 = const.tile([1, HD], FR)
        nc.vector.tensor_copy(ones64[:], onesf[:])

        def emit_proj(j):
            # QKV projections for q-chunk j; et=0 (head-pair 0) first for both
            # Q and K so attention hp=0 can begin before et=1 lands
            for et in range(2):
                for w_s, dst in ((wq, qT), (wk, kT)):
                    ps = pmm.tile([P, FD], F32, tag="mm", name="ps_proj")
                    for c in range(DC):
                        nc.tensor.matmul(
                            ps[:],
                            lhsT=w_s[:, c, ts(et, P)],
                            rhs=xT[:, c, ts(j, QC)],
                            start=(c == 0),
                            stop=(c == DC - 1),
                        )
                    nc.vector.tensor_copy(dst[:, et, ts(j, QC)], ps[:])
            for nt in range(4 * j, 4 * j + 4):
                psv = pmm.tile([P, FD], F32, tag="mm", name="ps_v")
                for c in range(DC):
                    nc.tensor.matmul(
                        psv[:, :E],
                        lhsT=xT[:, c, ts(nt, P)],
                        rhs=wv[:, c, :],
                        start=(c == 0),
                        stop=(c == DC - 1),
                    )
                nc.vector.tensor_copy(
                    vS[:, nt, :, 0:HD],
                    psv[:, :E].rearrange("p (h d) -> p h d", h=H4),
                )

        emit_proj(0)
        for j in range(NQ):
            # ---- attention for q-chunk j, heads processed in pairs ----
            nk = 4 * (j + 1)
            for hp in range(2):
                h0, h1 = 2 * hp, 2 * hp + 1
                pv0 = pacc.tile([HD + 1, QC], F32, tag="pv", name="pv0")
                pv1 = pacc.tile([HD + 1, QC], F32, tag="pv", name="pv1")
                for i in range(nk):
                    coff = P * (i - 4 * j)  # <0 for full (non-diagonal) tiles
                    cs = max(coff, 0)  # bf16 matmuls: full rate at any N
                    sc = psc.tile([P, 2, QC], F32, tag="sc", name="sc")
                    qs = slice(QC * j + cs, QC * (j + 1))
                    nc.tensor.matmul(
                        sc[:, 0, cs:QC],
                        lhsT=kT[0:HD, hp, ts(i, P)],
                        rhs=qT[0:HD, hp, qs],
                        start=True,
                        stop=True,
                    )
                    nc.tensor.matmul(
                        sc[:, 1, cs:QC],
                        lhsT=kT[HD:P, hp, ts(i, P)],
                        rhs=qT[HD:P, hp, qs],
                        start=True,
                        stop=True,
                    )
                    at = attn_pool.tile([P, 2, QC], BF, tag="at", name="at")
                    nc.scalar.activation(at[:, :, cs:QC], sc[:, :, cs:QC], EXP)
                    if coff >= 0:
                        # causal mask on the diagonal 128-block
                        for hh in range(2):
                            nc.gpsimd.tensor_tensor(
                                at[:, hh, coff:coff + P],
                                at[:, hh, coff:coff + P],
                                tri[:],
                                MUL,
                            )
                    pn = cs  # PV trim
                    for hh, pv in ((0, pv0), (1, pv1)):
                        nc.tensor.matmul(
                            pv[:, pn:QC],
                            lhsT=vS[:, i, 2 * hp + hh, :],
                            rhs=at[:, hh, pn:QC],
                            start=(i == 0),
                            stop=(i == nk - 1),
                            skip_group_check=True,
                        )
                # one 2-bank PSUM tile for both heads' rowsum broadcasts so
                # the other score slot stays free for the next i-loop;
                # per-head chains release each pacc bank as soon as possible
                bcp = psc.tile([P, 2, QC], F32, tag="sc", name="bcp")
                for hh, pv in ((0, pv0), (1, pv1)):
                    po = HD * hh
                    rec_fr = small.tile([1, QC], FR, tag="recf", name="rec_fr")
                    # f32r is full-width fp32; only the PE treats it specially
                    with nc.allow_low_precision(reason="f32r == fp32 width"):
                        nc.vector.reciprocal(rec_fr[:], pv[HD:HD + 1, :])
                    # broadcast 1/rowsum across 64 partitions via a rank-1
                    # ones-matmul (PE)
                    nc.tensor.matmul(
                        bcp[0:HD, hh, :], lhsT=ones64[:], rhs=rec_fr[:],
                        start=True, stop=True,
                    )
                    bc = small.tile([HD, QC], F32, tag="bc", name="bc")
                    nc.vector.tensor_copy(bc[:], bcp[0:HD, hh, :])
                    nc.vector.tensor_tensor(
                        cT[po:po + HD, hp, ts(j, QC)], pv[0:HD, :], bc[:], MUL
                    )

            # ---- next chunk's projections first: their PSUM slots queue
            # ahead of out-proj's, so the scheduler can slide them into the
            # ACT-bound attention gaps of this chunk ----
            if j + 1 < NQ:
                emit_proj(j + 1)

            # ---- out-projection for the finished n-chunk; the c=0 matmul
            # needs only head-pair 0's context, so it can run while hp=1 is
            # still in flight. Stage both fc halves into one tile so each
            # row block is a single DMA ----
            for nt in range(4 * j, 4 * j + 4):
                ob = ostage.tile([P, 2, FD], F32, tag="ob", name="ob")
                pos = []
                for fc in range(2):
                    po = pmm.tile([P, FD], F32, tag="mm", name="ps_out")
                    nc.tensor.matmul(
                        po[:], lhsT=cT[:, 0, ts(nt, P)],
                        rhs=wo[:, 0, ts(fc, FD)], start=True, stop=False,
                    )
                    pos.append(po)
                for fc in range(2):
                    nc.tensor.matmul(
                        pos[fc][:], lhsT=cT[:, 1, ts(nt, P)],
                        rhs=wo[:, 1, ts(fc, FD)], start=False, stop=True,
                    )
                    nc.vector.tensor_copy(ob[:, fc, :], pos[fc][:])
                nc.sync.dma_start(
                    out_d[ts(nt, P), :], ob[:].rearrange("p a b -> p (a b)")
                )


def _split_multi_waits(nc):
    """The TRN2 instruction encoding carries ONE sync-wait slot; this walrus
    build rejects instructions with more. Hoist extra waits onto standalone
    EventSemaphore instructions immediately before (same engine queue, same
    semantics)."""
    n = 0
    for f in nc.m.functions:
        for b in f.blocks:
            out = []
            for i in list(b.instructions):
                si = i.sync_info
                if si is not None and len(si.on_wait) > 1:
                    waits = list(si.on_wait)
                    for w in waits[:-1]:
                        n += 1
                        out.append(
                            mybir.InstEventSemaphore(
                                name=f"I-wsplit{n}",
                                engine=i.engine,
                                ins=[],
                                outs=[],
                                sync_info=mybir.SyncInfo(on_wait=[w], on_update=[]),
                            )
                        )
                    i.sync_info = mybir.SyncInfo(
                        on_wait=[waits[-1]], on_update=list(si.on_update)
                    )
                out.append(i)
            b.instructions = out


def declare_inputs(nc):
    xT_d = nc.dram_tensor("xT", [P, DC, S], BF, kind="ExternalInput").ap()
    wq_d = nc.dram_tensor("wqT", [P, DC, E], BF, kind="ExternalInput").ap()
    wk_d = nc.dram_tensor("wkT", [P, DC, E], BF, kind="ExternalInput").ap()
    wv_d = nc.dram_tensor("wvT", [P, DC, E], BF, kind="ExternalInput").ap()
    wo_d = nc.dram_tensor("woT", [P, 2, DD], FR, kind="ExternalInput").ap()
    msk_d = nc.dram_tensor("msk", [P, 2 * P], BF, kind="ExternalInput").ap()
    out_d = nc.dram_tensor("out", [S, DD], F32, kind="ExternalOutput").ap()
    return xT_d, wq_d, wk_d, wv_d, wo_d, msk_d, out_d


def build_nc(split_waits=True):
    nc = bass.Bass("TRN2", target_bir_lowering=False, debug=False)
    args = declare_inputs(nc)
    with tile.TileContext(nc) as tc:
        _emit(tc, nc, *args)
    if split_waits:
        _split_multi_waits(nc)
    return nc


def _strip(a, chunks):
    """[D, N] -> [128, D//128, N] with partition-major layout, contiguous."""
    d, n = a.shape
    return np.ascontiguousarray(
        a.reshape(chunks, P, n).transpose(1, 0, 2), dtype=np.float32
    )


import ml_dtypes  # noqa: E402


def make_in_maps(x, Wq, Wk, Wv, Wo):

    tri = np.triu(np.ones((P, P), np.float32))
    msk = np.concatenate([np.zeros((P, P), np.float32), tri], axis=1).astype(
        ml_dtypes.bfloat16
    )
    in_maps = []
    for c in range(8):
        b, g = c // 4, c % 4
        sl = slice(E * g, E * (g + 1))
        in_maps.append(
            {
                "xT": _strip(x[b].T.astype(np.float32), DC).astype(
                    ml_dtypes.bfloat16
                ),
                "wqT": _strip((Wq[sl, :] * 0.125).T.astype(np.float32), DC)
                .astype(ml_dtypes.bfloat16),
                "wkT": _strip(Wk[sl, :].T.astype(np.float32), DC).astype(
                    ml_dtypes.bfloat16
                ),
                "wvT": _strip(Wv[sl, :].T.astype(np.float32), DC).astype(
                    ml_dtypes.bfloat16
                ),
                "woT": _strip(Wo[:, sl].T.astype(np.float32), 2),
                "msk": msk,
            }
        )
    return in_maps


def kernel(x, Wq, Wk, Wv, Wo, bo, _run_kwargs=None):
    x, Wq, Wk, Wv, Wo, bo = (
        np.asarray(a, dtype=np.float32) for a in (x, Wq, Wk, Wv, Wo, bo)
    )
    nc = build_nc()
    in_maps = make_in_maps(x, Wq, Wk, Wv, Wo)
    res = run_bass_kernel_spmd(
        nc, in_maps, core_ids=list(range(8)), **(_run_kwargs or {})
    )
    out = np.zeros((2, S, DD), dtype=np.float32)
    for c in range(8):
        out[c // 4] += res.results[c]["out"]
    out += bo[None, None, :]
    if _run_kwargs:
        kernel.last_results = res
    return out


# revision 41
# speedup vs baseline: 1.1961x; 1.0940x over previous
"""Multi-head causal attention (B=2, S=2048, D=1024, H=16) on 8 TRN2 NeuronCores.

Sharding: core c -> batch c//4, head-quarter c%4 (4 heads = 256 head dims).
Each core runs the full pipeline for its (batch, 4 heads):
  QKV projections -> causal softmax(QK^T/8) -> PV -> partial out-projection.
Host pre-transposes x / weight shards (so every DMA is contiguous) and
sums the 4 row-sharded out-projection partials per batch + bias.

Engine split: PE matmuls (QKV projections / scores / PV in bf16 at full
PE rate, scores pair-tiled on rows 0-63/64-127 so each head pair runs
concurrently, out-projection fp32r), ACT exclusively exp (one activation
per (i, head-pair) spanning a 2-bank PSUM score tile), DVE all
PSUM->SBUF copies + softmax normalize, GpSimd(Pool) causal-mask
multiplies. Softmax skips max-subtraction (scores bounded ~ +-4); row
sums come free as a 65th ones-row in V. Diagonal blocks trim matmul N
exactly (bf16 streams full rate at any N). Emission order is pipelined:
chunk j+1's projections are emitted before chunk j's out-projection so
their PSUM slots queue first and the scheduler slides them into the
ACT-bound attention gaps; DMAs are batched (~0.7us SP issue cost each).
"""

import sys

import numpy as np

if "/opt/trn_rl_repo" not in sys.path:
    sys.path.insert(0, "/opt/trn_rl_repo")

import concourse.bass as bass
import concourse.mybir as mybir
import concourse.tile as tile
from concourse.bass import ts
from concourse.bass_utils import run_bass_kernel_spmd

P = 128          # partitions
S = 2048         # sequence length
DD = 1024        # model dim
DC = DD // P     # d-model chunks (8)
E = 256          # head dims per core (4 heads x 64)
H4 = 4           # heads per core
HD = 64
NQ = 4           # q chunks of 512
QC = 512
KT = S // P      # k tiles (16)
FD = 512         # matmul free dim

F32 = mybir.dt.float32
FR = mybir.dt.float32r
BF = mybir.dt.bfloat16
EXP = mybir.ActivationFunctionType.Exp
MUL = mybir.AluOpType.mult


def _emit(tc, nc, xT_d, wq_d, wk_d, wv_d, wo_d, msk_d, out_d):
    with (
        tc.tile_pool(name="const", bufs=1) as const,
        tc.tile_pool(name="attn", bufs=6) as attn_pool,
        tc.tile_pool(name="small", bufs=2) as small,
        tc.tile_pool(name="ostage", bufs=2) as ostage,
        tc.tile_pool(name="pmm", bufs=2, space="PSUM") as pmm,
        tc.tile_pool(name="pacc", bufs=2, space="PSUM") as pacc,
        tc.tile_pool(name="psc", bufs=2, space="PSUM") as psc,
    ):
        xT = const.tile([P, DC, S], BF)
        wq = const.tile([P, DC, E], BF)
        wk = const.tile([P, DC, E], BF)
        wv = const.tile([P, DC, E], BF)
        wo = const.tile([P, 2, DD], FR)
        msk = const.tile([P, 2 * P], BF)  # [zeros(128) | upper-tri(128)]
        qT = const.tile([P, 2, S], BF)
        kT = const.tile([P, 2, S], BF)
        vS = const.tile([P, KT, H4, HD + 1], BF)
        cT = const.tile([P, 2, S], FR)

        # DMA issue on SP costs ~0.7us per dma_start, so batch: halves of
        # (wq, xT j=0) first so the first projection matmuls start early
        nc.sync.dma_start(wq[:, 0:4], wq_d[:, 0:4])
        nc.sync.dma_start(xT[:, 0:4, ts(0, QC)], xT_d[:, 0:4, ts(0, QC)])
        nc.sync.dma_start(wq[:, 4:8], wq_d[:, 4:8])
        nc.sync.dma_start(xT[:, 4:8, ts(0, QC)], xT_d[:, 4:8, ts(0, QC)])
        nc.sync.dma_start(wk[:], wk_d[:])
        nc.sync.dma_start(msk[:], msk_d[:])
        nc.sync.dma_start(wv[:], wv_d[:])
        # remaining x^T per q-chunk
        for j in range(1, NQ):
            nc.sync.dma_start(xT[:, :, ts(j, QC)], xT_d[:, :, ts(j, QC)])
        nc.sync.dma_start(wo[:], wo_d[:])

        tri = msk[:, P:]        # [128, 128] upper-tri ones (q >= k)

        # ones column of V_ext (row sums of exp-scores come out of PV)
        nc.gpsimd.memset(vS[:, :, :, HD], 1.0)
        # f32r ones row for the rowsum-broadcast matmul (Memset can't encode
        # f32r; a copy can round to it)
        onesf = const.tile([1, HD], F32)
        nc.vector.memset(onesf[:], 1.0)
        ones64 = const.tile([1, HD], FR)
        nc.vector.tensor_copy(ones64[:], onesf[:])

        def emit_proj(j):
            # QKV projections for q-chunk j; et=0 (head-pair 0) first for both
            # Q and K so attention hp=0 can begin before et=1 lands
            for et in range(2):
                for w_s, dst in ((wq, qT), (wk, kT)):
                    ps = pmm.tile([P, FD], F32, tag="mm", name="ps_proj")
                    for c in range(DC):
                        nc.tensor.matmul(
                            ps[:],
                            lhsT=w_s[:, c, ts(et, P)],
                            rhs=xT[:, c, ts(j, QC)],
                            start=(c == 0),
                            stop=(c == DC - 1),
                        )
                    nc.vector.tensor_copy(dst[:, et, ts(j, QC)], ps[:])
            for nt in range(4 * j, 4 * j + 4):
                psv = pmm.tile([P, FD], F32, tag="mm", name="ps_v")
                for c in range(DC):
                    nc.tensor.matmul(
                        psv[:, :E],
                        lhsT=xT[:, c, ts(nt, P)],
                        rhs=wv[:, c, :],
                        start=(c == 0),
                        stop=(c == DC - 1),
                    )
                nc.vector.tensor_copy(
                    vS[:, nt, :, 0:HD],
                    psv[:, :E].rearrange("p (h d) -> p h d", h=H4),
                )

        emit_proj(0)
        for j in range(NQ):
            # ---- attention for q-chunk j, heads processed in pairs ----
            nk = 4 * (j + 1)
            for hp in range(2):
                pv0 = pacc.tile([HD + 1, QC], F32, tag="pv", name="pv0")
                pv1 = pacc.tile([HD + 1, QC], F32, tag="pv", name="pv1")
                for i in range(nk):
                    coff = P * (i - 4 * j)  # <0 for full (non-diagonal) tiles
                    cs = max(coff, 0)  # bf16 matmuls: full rate at any N
                    sc = psc.tile([P, 2, QC], F32, tag="sc", name="sc")
                    qs = slice(QC * j + cs, QC * (j + 1))
                    nc.tensor.matmul(
                        sc[:, 0, cs:QC],
                        lhsT=kT[0:HD, hp, ts(i, P)],
                        rhs=qT[0:HD, hp, qs],
                        start=True,
                        stop=True,
                    )
                    nc.tensor.matmul(
                        sc[:, 1, cs:QC],
                        lhsT=kT[HD:P, hp, ts(i, P)],
                        rhs=qT[HD:P, hp, qs],
                        start=True,
                        stop=True,
                    )
                    at = attn_pool.tile([P, 2, QC], BF, tag="at", name="at")
                    nc.scalar.activation(at[:, :, cs:QC], sc[:, :, cs:QC], EXP)
                    if coff >= 0:
                        # causal mask on the diagonal 128-block
                        for hh in range(2):
                            nc.gpsimd.tensor_tensor(
                                at[:, hh, coff:coff + P],
                                at[:, hh, coff:coff + P],
                                tri[:],
                                MUL,
                            )
                    pn = cs  # PV trim
                    for hh, pv in ((0, pv0), (1, pv1)):
                        nc.tensor.matmul(
                            pv[:, pn:QC],
                            lhsT=vS[:, i, 2 * hp + hh, :],
                            rhs=at[:, hh, pn:QC],
                            start=(i == 0),
                            stop=(i == nk - 1),
                            skip_group_check=True,
                        )
                # one 2-bank PSUM tile for both heads' rowsum broadcasts so
                # the other score slot stays free for the next i-loop;
                # per-head chains release each pacc bank as soon as possible
                bcp = psc.tile([P, 2, QC], F32, tag="sc", name="bcp")
                for hh, pv in ((0, pv0), (1, pv1)):
                    po = HD * hh
                    rec_fr = small.tile([1, QC], FR, tag="recf", name="rec_fr")
                    # f32r is full-width fp32; only the PE treats it specially
                    with nc.allow_low_precision(reason="f32r == fp32 width"):
                        nc.vector.reciprocal(rec_fr[:], pv[HD:HD + 1, :])
                    # broadcast 1/rowsum across 64 partitions via a rank-1
                    # ones-matmul (PE)
                    nc.tensor.matmul(
                        bcp[0:HD, hh, :], lhsT=ones64[:], rhs=rec_fr[:],
                        start=True, stop=True,
                    )
                    bc = small.tile([HD, QC], F32, tag="bc", name="bc")
                    nc.vector.tensor_copy(bc[:], bcp[0:HD, hh, :])
                    nc.vector.tensor_tensor(
                        cT[po:po + HD, hp, ts(j, QC)], pv[0:HD, :], bc[:], MUL
                    )

            # ---- next chunk's projections first: their PSUM slots queue
            # ahead of out-proj's, so the scheduler can slide them into the
            # ACT-bound attention gaps of this chunk ----
            if j + 1 < NQ:
                emit_proj(j + 1)

            # ---- out-projection for the finished n-chunk; the c=0 matmul
            # needs only head-pair 0's context, so it can run while hp=1 is
            # still in flight. Stage both fc halves into one tile so each
            # row block is a single DMA ----
            for nt in range(4 * j, 4 * j + 4):
                ob = ostage.tile([P, 2, FD], F32, tag="ob", name="ob")
                pos = []
                for fc in range(2):
                    po = pmm.tile([P, FD], F32, tag="mm", name="ps_out")
                    nc.tensor.matmul(
                        po[:], lhsT=cT[:, 0, ts(nt, P)],
                        rhs=wo[:, 0, ts(fc, FD)], start=True, stop=False,
                    )
                    pos.append(po)
                for fc in range(2):
                    nc.tensor.matmul(
                        pos[fc][:], lhsT=cT[:, 1, ts(nt, P)],
                        rhs=wo[:, 1, ts(fc, FD)], start=False, stop=True,
                    )
                    nc.vector.tensor_copy(ob[:, fc, :], pos[fc][:])
                nc.sync.dma_start(
                    out_d[ts(nt, P), :], ob[:].rearrange("p a b -> p (a b)")
                )


def _split_multi_waits(nc):
    """The TRN2 instruction encoding carries ONE sync-wait slot; this walrus
    build rejects instructions with more. Hoist extra waits onto standalone
    EventSemaphore instructions immediately before (same engine queue, same
    semantics)."""
    n = 0
    for f in nc.m.functions:
        for b in f.blocks:
            out = []
            for i in list(b.instructions):
                si = i.sync_info
                if si is not None and len(si.on_wait) > 1:
                    waits = list(si.on_wait)
                    for w in waits[:-1]:
                        n += 1
                        out.append(
                            mybir.InstEventSemaphore(
                                name=f"I-wsplit{n}",
                                engine=i.engine,
                                ins=[],
                                outs=[],
                                sync_info=mybir.SyncInfo(on_wait=[w], on_update=[]),
                            )
                        )
                    i.sync_info = mybir.SyncInfo(
                        on_wait=[waits[-1]], on_update=list(si.on_update)
                    )
                out.append(i)
            b.instructions = out


def declare_inputs(nc):
    xT_d = nc.dram_tensor("xT", [P, DC, S], BF, kind="ExternalInput").ap()
    wq_d = nc.dram_tensor("wqT", [P, DC, E], BF, kind="ExternalInput").ap()
    wk_d = nc.dram_tensor("wkT", [P, DC, E], BF, kind="ExternalInput").ap()
    wv_d = nc.dram_tensor("wvT", [P, DC, E], BF, kind="ExternalInput").ap()
    wo_d = nc.dram_tensor("woT", [P, 2, DD], FR, kind="ExternalInput").ap()
    msk_d = nc.dram_tensor("msk", [P, 2 * P], BF, kind="ExternalInput").ap()
    out_d = nc.dram_tensor("out", [S, DD], F32, kind="ExternalOutput").ap()
    return xT_d, wq_d, wk_d, wv_d, wo_d, msk_d, out_d


def build_nc(split_waits=True):
    nc = bass.Bass("TRN2", target_bir_lowering=False, debug=False)
    args = declare_inputs(nc)
    with tile.TileContext(nc) as tc:
        _emit(tc, nc, *args)
    if split_waits:
        _split_multi_waits(nc)
    return nc


def _strip(a, chunks):
    """[D, N] -> [128, D//128, N] with partition-major layout, contiguous."""
    d, n = a.shape
    return np.ascontiguousarray(
        a.reshape(chunks, P, n).transpose(1, 0, 2), dtype=np.float32
    )


import ml_dtypes  # noqa: E402


def make_in_maps(x, Wq, Wk, Wv, Wo):

    tri = np.triu(np.ones((P, P), np.float32))
    msk = np.concatenate([np.zeros((P, P), np.float32), tri], axis=1).astype(
        ml_dtypes.bfloat16
    )
    in_maps = []
    for c in range(8):
        b, g = c // 4, c % 4
        sl = slice(E * g, E * (g + 1))
        in_maps.append(
            {
                "xT": _strip(x[b].T.astype(np.float32), DC).astype(
                    ml_dtypes.bfloat16
                ),
                "wqT": _strip((Wq[sl, :] * 0.125).T.astype(np.float32), DC)
                .astype(ml_dtypes.bfloat16),
                "wkT": _strip(Wk[sl, :].T.astype(np.float32), DC).astype(
                    ml_dtypes.bfloat16
                ),
                "wvT": _strip(Wv[sl, :].T.astype(np.float32), DC).astype(
                    ml_dtypes.bfloat16
                ),
                "woT": _strip(Wo[:, sl].T.astype(np.float32), 2),
                "msk": msk,
            }
        )
    return in_maps


def kernel(x, Wq, Wk, Wv, Wo, bo, _run_kwargs=None):
    x, Wq, Wk, Wv, Wo, bo = (
        np.asarray(a, dtype=np.float32) for a in (x, Wq, Wk, Wv, Wo, bo)
    )
    nc = build_nc()
    in_maps = make_in_maps(x, Wq, Wk, Wv, Wo)
    res = run_bass_kernel_spmd(
        nc, in_maps, core_ids=list(range(8)), **(_run_kwargs or {})
    )
    out = np.zeros((2, S, DD), dtype=np.float32)
    for c in range(8):
        out[c // 4] += res.results[c]["out"]
    out += bo[None, None, :]
    if _run_kwargs:
        kernel.last_results = res
    return out


# revision 44
# speedup vs baseline: 1.2803x; 1.0704x over previous
"""Multi-head causal attention (B=2, S=2048, D=1024, H=16) on 8 TRN2 NeuronCores.

Sharding: core c -> batch c//4, head-quarter c%4 (4 heads = 256 head dims).
Each core runs the full pipeline for its (batch, 4 heads):
  QKV projections -> causal softmax(QK^T/8) -> PV -> partial out-projection.
Host pre-transposes x / weight shards (so every DMA is contiguous) and
sums the 4 row-sharded out-projection partials per batch + bias.

Engine split: PE matmuls (QKV projections / scores / PV in bf16 at full
PE rate, scores pair-tiled on rows 0-63/64-127 so each head pair runs
concurrently, out-projection fp32r), ACT exclusively exp (one activation
per (i, head-pair) spanning a 2-bank PSUM score tile), DVE all
PSUM->SBUF copies + softmax normalize, GpSimd(Pool) causal-mask
multiplies. Softmax skips max-subtraction (scores bounded ~ +-4); row
sums come free as a 65th ones-row in V. Diagonal blocks trim matmul N
exactly (bf16 streams full rate at any N). Emission order is pipelined:
chunk j+1's projections are emitted before chunk j's out-projection so
their PSUM slots queue first and the scheduler slides them into the
ACT-bound attention gaps; DMAs are batched (~0.7us SP issue cost each).
"""

import sys

import numpy as np

if "/opt/trn_rl_repo" not in sys.path:
    sys.path.insert(0, "/opt/trn_rl_repo")

import concourse.bass as bass
import concourse.mybir as mybir
import concourse.tile as tile
from concourse.bass import ts
from concourse.bass_utils import run_bass_kernel_spmd

P = 128          # partitions
S = 2048         # sequence length
DD = 1024        # model dim
DC = DD // P     # d-model chunks (8)
E = 256          # head dims per core (4 heads x 64)
H4 = 4           # heads per core
HD = 64
NQ = 4           # q chunks of 512
QC = 512
KT = S // P      # k tiles (16)
FD = 512         # matmul free dim

F32 = mybir.dt.float32
FR = mybir.dt.float32r
BF = mybir.dt.bfloat16
EXP = mybir.ActivationFunctionType.Exp
MUL = mybir.AluOpType.mult


def _emit(tc, nc, xT_d, wq_d, wk_d, wv_d, wo_d, msk_d, out_d):
    with (
        tc.tile_pool(name="const", bufs=1) as const,
        tc.tile_pool(name="attn", bufs=8) as attn_pool,
        tc.tile_pool(name="small", bufs=8) as small,
        tc.tile_pool(name="ostage", bufs=6) as ostage,
        tc.tile_pool(name="pmm", bufs=2, space="PSUM") as pmm,
        tc.tile_pool(name="pacc", bufs=2, space="PSUM") as pacc,
        tc.tile_pool(name="psc", bufs=2, space="PSUM") as psc,
    ):
        xT = const.tile([P, DC, S], BF)
        wq = const.tile([P, DC, E], BF)
        wk = const.tile([P, DC, E], BF)
        wv = const.tile([P, DC, E], BF)
        wo = const.tile([P, 2, DD], FR)
        msk = const.tile([P, 2 * P], BF)  # [zeros(128) | upper-tri(128)]
        qT = const.tile([P, 2, S], BF)
        kT = const.tile([P, 2, S], BF)
        vS = const.tile([P, KT, H4, HD + 1], BF)
        cT = const.tile([P, 2, S], FR)

        # DMA issue on SP costs ~0.7us per dma_start, so batch: halves of
        # (wq, xT j=0) first so the first projection matmuls start early
        nc.sync.dma_start(wq[:, 0:4], wq_d[:, 0:4])
        nc.sync.dma_start(xT[:, 0:4, ts(0, QC)], xT_d[:, 0:4, ts(0, QC)])
        nc.sync.dma_start(wq[:, 4:8], wq_d[:, 4:8])
        nc.sync.dma_start(xT[:, 4:8, ts(0, QC)], xT_d[:, 4:8, ts(0, QC)])
        nc.sync.dma_start(wk[:], wk_d[:])
        nc.sync.dma_start(msk[:], msk_d[:])
        nc.sync.dma_start(wv[:], wv_d[:])
        # remaining x^T per q-chunk
        for j in range(1, NQ):
            nc.sync.dma_start(xT[:, :, ts(j, QC)], xT_d[:, :, ts(j, QC)])
        nc.sync.dma_start(wo[:], wo_d[:])

        tri = msk[:, P:]        # [128, 128] upper-tri ones (q >= k)

        # ones column of V_ext (row sums of exp-scores come out of PV)
        nc.gpsimd.memset(vS[:, :, :, HD], 1.0)
        # f32r ones row for the rowsum-broadcast matmul (Memset can't encode
        # f32r; a copy can round to it)
        onesf = const.tile([1, HD], F32)
        nc.vector.memset(onesf[:], 1.0)
        ones64 = const.tile([1, HD], FR)
        nc.vector.tensor_copy(ones64[:], onesf[:])

        def emit_proj(j):
            # QKV projections for q-chunk j; et=0 (head-pair 0) first for both
            # Q and K so attention hp=0 can begin before et=1 lands
            for et in range(2):
                for w_s, dst in ((wq, qT), (wk, kT)):
                    ps = pmm.tile([P, FD], F32, tag="mm", name="ps_proj")
                    for c in range(DC):
                        nc.tensor.matmul(
                            ps[:],
                            lhsT=w_s[:, c, ts(et, P)],
                            rhs=xT[:, c, ts(j, QC)],
                            start=(c == 0),
                            stop=(c == DC - 1),
                        )
                    nc.vector.tensor_copy(dst[:, et, ts(j, QC)], ps[:])
            for nt in range(4 * j, 4 * j + 4):
                psv = pmm.tile([P, FD], F32, tag="mm", name="ps_v")
                for c in range(DC):
                    nc.tensor.matmul(
                        psv[:, :E],
                        lhsT=xT[:, c, ts(nt, P)],
                        rhs=wv[:, c, :],
                        start=(c == 0),
                        stop=(c == DC - 1),
                    )
                nc.vector.tensor_copy(
                    vS[:, nt, :, 0:HD],
                    psv[:, :E].rearrange("p (h d) -> p h d", h=H4),
                )

        emit_proj(0)
        for j in range(NQ):
            # ---- attention for q-chunk j, heads processed in pairs ----
            nk = 4 * (j + 1)
            for hp in range(2):
                pv0 = pacc.tile([HD + 1, QC], F32, tag="pv", name="pv0")
                pv1 = pacc.tile([HD + 1, QC], F32, tag="pv", name="pv1")
                for i in range(nk):
                    coff = P * (i - 4 * j)  # <0 for full (non-diagonal) tiles
                    cs = max(coff, 0)  # bf16 matmuls: full rate at any N
                    sc = psc.tile([P, 2, QC], F32, tag="sc", name="sc")
                    qs = slice(QC * j + cs, QC * (j + 1))
                    nc.tensor.matmul(
                        sc[:, 0, cs:QC],
                        lhsT=kT[0:HD, hp, ts(i, P)],
                        rhs=qT[0:HD, hp, qs],
                        start=True,
                        stop=True,
                    )
                    nc.tensor.matmul(
                        sc[:, 1, cs:QC],
                        lhsT=kT[HD:P, hp, ts(i, P)],
                        rhs=qT[HD:P, hp, qs],
                        start=True,
                        stop=True,
                    )
                    at = attn_pool.tile([P, 2, QC], BF, tag="at", name="at")
                    nc.scalar.activation(at[:, :, cs:QC], sc[:, :, cs:QC], EXP)
                    if coff >= 0:
                        # causal mask on the diagonal 128-block
                        for hh in range(2):
                            nc.gpsimd.tensor_tensor(
                                at[:, hh, coff:coff + P],
                                at[:, hh, coff:coff + P],
                                tri[:],
                                MUL,
                            )
                    pn = cs  # PV trim
                    for hh, pv in ((0, pv0), (1, pv1)):
                        nc.tensor.matmul(
                            pv[:, pn:QC],
                            lhsT=vS[:, i, 2 * hp + hh, :],
                            rhs=at[:, hh, pn:QC],
                            start=(i == 0),
                            stop=(i == nk - 1),
                            skip_group_check=True,
                        )
                # one 2-bank PSUM tile for both heads' rowsum broadcasts so
                # the other score slot stays free for the next i-loop;
                # per-head chains release each pacc bank as soon as possible
                bcp = psc.tile([P, 2, QC], F32, tag="sc", name="bcp")
                for hh, pv in ((0, pv0), (1, pv1)):
                    po = HD * hh
                    rec_fr = small.tile([1, QC], FR, tag="recf", name="rec_fr")
                    # f32r is full-width fp32; only the PE treats it specially
                    with nc.allow_low_precision(reason="f32r == fp32 width"):
                        nc.vector.reciprocal(rec_fr[:], pv[HD:HD + 1, :])
                    # broadcast 1/rowsum across 64 partitions via a rank-1
                    # ones-matmul (PE)
                    nc.tensor.matmul(
                        bcp[0:HD, hh, :], lhsT=ones64[:], rhs=rec_fr[:],
                        start=True, stop=True,
                    )
                    bc = small.tile([HD, QC], F32, tag="bc", name="bc")
                    nc.vector.tensor_copy(bc[:], bcp[0:HD, hh, :])
                    nc.vector.tensor_tensor(
                        cT[po:po + HD, hp, ts(j, QC)], pv[0:HD, :], bc[:], MUL
                    )

            # ---- next chunk's projections first: their PSUM slots queue
            # ahead of out-proj's, so the scheduler can slide them into the
            # ACT-bound attention gaps of this chunk ----
            if j + 1 < NQ:
                emit_proj(j + 1)

            # ---- out-projection for the finished n-chunk; the c=0 matmul
            # needs only head-pair 0's context, so it can run while hp=1 is
            # still in flight. Stage both fc halves into one tile so each
            # row block is a single DMA ----
            for nt in range(4 * j, 4 * j + 4):
                ob = ostage.tile([P, 2, FD], F32, tag="ob", name="ob")
                pos = []
                for fc in range(2):
                    po = pmm.tile([P, FD], F32, tag="mm", name="ps_out")
                    nc.tensor.matmul(
                        po[:], lhsT=cT[:, 0, ts(nt, P)],
                        rhs=wo[:, 0, ts(fc, FD)], start=True, stop=False,
                    )
                    pos.append(po)
                for fc in range(2):
                    nc.tensor.matmul(
                        pos[fc][:], lhsT=cT[:, 1, ts(nt, P)],
                        rhs=wo[:, 1, ts(fc, FD)], start=False, stop=True,
                    )
                    nc.vector.tensor_copy(ob[:, fc, :], pos[fc][:])
                nc.sync.dma_start(
                    out_d[ts(nt, P), :], ob[:].rearrange("p a b -> p (a b)")
                )


def _split_multi_waits(nc):
    """The TRN2 instruction encoding carries ONE sync-wait slot; this walrus
    build rejects instructions with more. Hoist extra waits onto standalone
    EventSemaphore instructions immediately before (same engine queue, same
    semantics)."""
    n = 0
    for f in nc.m.functions:
        for b in f.blocks:
            out = []
            for i in list(b.instructions):
                si = i.sync_info
                if si is not None and len(si.on_wait) > 1:
                    waits = list(si.on_wait)
                    for w in waits[:-1]:
                        n += 1
                        out.append(
                            mybir.InstEventSemaphore(
                                name=f"I-wsplit{n}",
                                engine=i.engine,
                                ins=[],
                                outs=[],
                                sync_info=mybir.SyncInfo(on_wait=[w], on_update=[]),
                            )
                        )
                    i.sync_info = mybir.SyncInfo(
                        on_wait=[waits[-1]], on_update=list(si.on_update)
                    )
                out.append(i)
            b.instructions = out


def declare_inputs(nc):
    xT_d = nc.dram_tensor("xT", [P, DC, S], BF, kind="ExternalInput").ap()
    wq_d = nc.dram_tensor("wqT", [P, DC, E], BF, kind="ExternalInput").ap()
    wk_d = nc.dram_tensor("wkT", [P, DC, E], BF, kind="ExternalInput").ap()
    wv_d = nc.dram_tensor("wvT", [P, DC, E], BF, kind="ExternalInput").ap()
    wo_d = nc.dram_tensor("woT", [P, 2, DD], FR, kind="ExternalInput").ap()
    msk_d = nc.dram_tensor("msk", [P, 2 * P], BF, kind="ExternalInput").ap()
    out_d = nc.dram_tensor("out", [S, DD], F32, kind="ExternalOutput").ap()
    return xT_d, wq_d, wk_d, wv_d, wo_d, msk_d, out_d


def build_nc(split_waits=True):
    nc = bass.Bass("TRN2", target_bir_lowering=False, debug=False)
    args = declare_inputs(nc)
    with tile.TileContext(nc) as tc:
        _emit(tc, nc, *args)
    if split_waits:
        _split_multi_waits(nc)
    return nc


def _strip(a, chunks):
    """[D, N] -> [128, D//128, N] with partition-major layout, contiguous."""
    d, n = a.shape
    return np.ascontiguousarray(
        a.reshape(chunks, P, n).transpose(1, 0, 2), dtype=np.float32
    )


import ml_dtypes  # noqa: E402


def make_in_maps(x, Wq, Wk, Wv, Wo):

    tri = np.triu(np.ones((P, P), np.float32))
    msk = np.concatenate([np.zeros((P, P), np.float32), tri], axis=1).astype(
        ml_dtypes.bfloat16
    )
    in_maps = []
    for c in range(8):
        b, g = c // 4, c % 4
        sl = slice(E * g, E * (g + 1))
        in_maps.append(
            {
                "xT": _strip(x[b].T.astype(np.float32), DC).astype(
                    ml_dtypes.bfloat16
                ),
                "wqT": _strip((Wq[sl, :] * 0.125).T.astype(np.float32), DC)
                .astype(ml_dtypes.bfloat16),
                "wkT": _strip(Wk[sl, :].T.astype(np.float32), DC).astype(
                    ml_dtypes.bfloat16
                ),
                "wvT": _strip(Wv[sl, :].T.astype(np.float32), DC).astype(
                    ml_dtypes.bfloat16
                ),
                "woT": _strip(Wo[:, sl].T.astype(np.float32), 2),
                "msk": msk,
            }
        )
    return in_maps


def kernel(x, Wq, Wk, Wv, Wo, bo, _run_kwargs=None):
    x, Wq, Wk, Wv, Wo, bo = (
        np.asarray(a, dtype=np.float32) for a in (x, Wq, Wk, Wv, Wo, bo)
    )
    nc = build_nc()
    in_maps = make_in_maps(x, Wq, Wk, Wv, Wo)
    res = run_bass_kernel_spmd(
        nc, in_maps, core_ids=list(range(8)), **(_run_kwargs or {})
    )
    out = np.zeros((2, S, DD), dtype=np.float32)
    for c in range(8):
        out[c // 4] += res.results[c]["out"]
    out += bo[None, None, :]
    if _run_kwargs:
        kernel.last_results = res
    return out
